# revision 1
# baseline (speedup 1.0000x reference)
"""BERT self-attention (B=4, S=2048, D=1024, H=16) on 8 Trainium2 NeuronCores.

Tensor-parallel (Megatron) over heads: core c owns heads 2c, 2c+1.
  - Wq/Wk/Wv column-sharded (128 output dims per core), Wo row-sharded.
  - Each core consumes the full x, produces a partial (8192, 1024) output;
    partials are summed on the host (the Wo contraction over d_model is
    split across cores), plus bo.

Per-core dataflow (matmul inputs bf16, fp32 PSUM accumulate; bf16 cuts
the measured ~2x per-matmul HW overhead of 4-byte operands — LDWEIGHTS
fast-weight-load + faster streaming — and halves SBUF/DMA traffic):
  xT (1024, 8192) streamed in 512-token blocks
    -> Q,K (dq 128, tok 8192) dk-major   [lhsT=WqT/WkT k-tiles, rhs=xT]
    -> V (dv 128, tok 8192), PE-transposed per 128-tok tile into
       vt [tok 128, 130] = [Vh0 64 | ones | Vh1 64 | ones]
  scores.T tile [ktok 128, q 1024] = both heads' [*, 512] halves
    (row-tiled K=64 matmul pair at base partitions 0/64 -> runs
    CONCURRENTLY in the PE array, verified on HW; two PSUM banks)
  exp on ScalarE, one [128, 1024] op per ktok tile covering both heads
    (ScalarE is the bottleneck engine: amortize the ~352-cycle startup)
  ctx.T accumulation over 16 ktok tiles: lhsT=vt[:, h*65:(h+1)*65]
    (M=65: row 64 accumulates the softmax denominators for free)
  normalize: reciprocal of row 64 -> PE outer-product broadcast -> DVE mul
  out partial [tok 128, 512] = lhsT=ctxn tok-tile, rhs=WoT; PSUM->SBUF
    copies on DVE (not ScalarE), partials written bf16

Schedule: per-batch software pipeline — QKV token-blocks and V-transposes
of batch b+1 are emitted round-robin between the attention q-blocks of
batch b, so the PE always has dependency-free work while attention waits
on ScalarE exp.
"""
import sys

if "/opt/trn_rl_repo" not in sys.path:
    sys.path.insert(0, "/opt/trn_rl_repo")

import numpy as np

import concourse.bacc as bacc
import concourse.mybir as mybir
import concourse.tile as tile
from concourse.bass_utils import run_bass_kernel_spmd

DT = mybir.dt
AF = mybir.ActivationFunctionType

B, S, D, H = 4, 2048, 1024, 16
DK = D // H  # 64
NCORES = 8
HPC = H // NCORES  # heads per core = 2
DPC = HPC * DK  # output dims per core = 128
T = B * S  # 8192 tokens
TB = 512  # token block for projections
QB = 512  # query block for attention
NKT = S // 128  # 16 key tiles per sequence
NDT = D // 128  # 8 contraction tiles for projections

_cache = {}


def _build(with_mask, phase="full", nb=B, reps=1):
    nc = bacc.Bacc("TRN2", target_bir_lowering=False, debug=False)
    xT_d = nc.declare_dram_parameter("xT", [D, T], DT.bfloat16, isOutput=False)
    wq_d = nc.declare_dram_parameter("wqT", [D, DPC], DT.bfloat16, isOutput=False)
    wk_d = nc.declare_dram_parameter("wkT", [D, DPC], DT.bfloat16, isOutput=False)
    wv_d = nc.declare_dram_parameter("wvT", [D, DPC], DT.bfloat16, isOutput=False)
    wo_d = nc.declare_dram_parameter("woT", [DPC, D], DT.bfloat16, isOutput=False)
    bq_d = nc.declare_dram_parameter("bq", [DPC, 1], DT.float32, isOutput=False)
    bk_d = nc.declare_dram_parameter("bk", [DPC, 1], DT.float32, isOutput=False)
    bv_d = nc.declare_dram_parameter("bv", [DPC, 1], DT.float32, isOutput=False)
    id_d = nc.declare_dram_parameter("ident", [128, 128], DT.bfloat16, isOutput=False)
    if with_mask:
        mb_d = nc.declare_dram_parameter("mbias", [B, NKT, 128], DT.float32, isOutput=False)
    out_d = nc.declare_dram_parameter("out", [T, D], DT.bfloat16, isOutput=True)
    if phase == "qkv":
        dbg_d = nc.declare_dram_parameter("dbg", [3, 128, T], DT.bfloat16, isOutput=True)

    with tile.TileContext(nc) as tc:
        with (
            tc.tile_pool(name="cst", bufs=1) as cst,
            tc.tile_pool(name="qkv", bufs=1) as qkv,
            tc.tile_pool(name="xt", bufs=24) as xtp,
            tc.tile_pool(name="vt", bufs=32) as vtp,
            tc.tile_pool(name="es", bufs=8) as esp,
            tc.tile_pool(name="cn", bufs=6) as cnp,
            tc.tile_pool(name="os", bufs=6) as osp,
            tc.tile_pool(name="sm", bufs=4) as smp,
            tc.tile_pool(name="sps", bufs=2, space="PSUM") as sps,
            tc.tile_pool(name="cps", bufs=1, space="PSUM") as cps,
            tc.tile_pool(name="pmm", bufs=2, space="PSUM") as pmm,
        ):
            # ---- constants / weights ----
            wq = cst.tile([128, NDT, DPC], DT.bfloat16, tag="wq")
            wk = cst.tile([128, NDT, DPC], DT.bfloat16, tag="wk")
            wv = cst.tile([128, NDT, DPC], DT.bfloat16, tag="wv")
            nc.sync.dma_start(wq[:], wq_d.rearrange("(a p) m -> p a m", p=128))
            nc.sync.dma_start(wk[:], wk_d.rearrange("(a p) m -> p a m", p=128))
            nc.sync.dma_start(wv[:], wv_d.rearrange("(a p) m -> p a m", p=128))
            wo = cst.tile([DPC, D], DT.bfloat16, tag="wo")
            nc.sync.dma_start(wo[:], wo_d[:])
            bq = cst.tile([DPC, 1], DT.float32, tag="bq")
            bk = cst.tile([DPC, 1], DT.float32, tag="bk")
            bv = cst.tile([DPC, 1], DT.float32, tag="bv")
            nc.sync.dma_start(bq[:], bq_d[:])
            nc.sync.dma_start(bk[:], bk_d[:])
            nc.sync.dma_start(bv[:], bv_d[:])
            ident = cst.tile([128, 128], DT.bfloat16, tag="ident")
            nc.sync.dma_start(ident[:], id_d[:])
            ones128 = cst.tile([128, 1], DT.float32, tag="ones128")
            nc.vector.memset(ones128[:], 1.0)
            ones2 = cst.tile([128, 2], DT.float32, tag="ones2")
            nc.vector.memset(ones2[:], 1.0)
            onesc_f = cst.tile([1, 128], DT.float32, tag="onescf")
            nc.vector.memset(onesc_f[:, 0:64], 1.0)
            nc.vector.memset(onesc_f[:, 64:128], 0.0)
            onesh = cst.tile([1, 2, 128], DT.float32r, tag="onesh")
            nc.vector.tensor_copy(onesh[:, 0, :], onesc_f[:])
            nc.vector.tensor_copy(onesh[:, 1, 0:64], onesc_f[:, 64:128])
            nc.vector.tensor_copy(onesh[:, 1, 64:128], onesc_f[:, 0:64])
            if with_mask:
                mb = cst.tile([128, B, NKT], DT.float32, tag="mb")
                nc.sync.dma_start(mb[:], mb_d.rearrange("b a p -> p b a"))

            # persistent activations (dk/dv-major), one tile per batch so
            # attention for batch b only depends on batch b's projections
            q_sb = [qkv.tile([128, S], DT.bfloat16, tag=f"q{b}", name=f"q{b}") for b in range(B)]
            k_sb = [qkv.tile([128, S], DT.bfloat16, tag=f"k{b}", name=f"k{b}") for b in range(B)]
            v_sb = [qkv.tile([128, S], DT.bfloat16, tag=f"v{b}", name=f"v{b}") for b in range(B)]

            def emit_qkv(rep, b, blocks=None):
                """QKV projections for batch b (token blocks 4b..4b+3)."""
                for bc_i in blocks if blocks is not None else range(S // TB):
                    tb = b * (S // TB) + bc_i
                    xts = []
                    for dt_i in range(NDT):
                        xt = xtp.tile([128, TB], DT.bfloat16, tag="xt", name=f"{rep}_xt{tb}_{dt_i}")
                        nc.sync.dma_start(
                            xt[:],
                            xT_d[dt_i * 128 : (dt_i + 1) * 128, tb * TB : (tb + 1) * TB],
                        )
                        xts.append(xt)
                    for pname, w, bias, dst in (
                        ("q", wq, bq, q_sb),
                        ("k", wk, bk, k_sb),
                        ("v", wv, bv, v_sb),
                    ):
                        acc = pmm.tile([128, TB], DT.float32, tag="pmm", name=f"{rep}_p{pname}{tb}")
                        for dt_i in range(NDT):
                            nc.tensor.matmul(
                                acc[:],
                                w[:, dt_i, :],
                                xts[dt_i][:],
                                start=(dt_i == 0),
                                stop=(dt_i == NDT - 1),
                            )
                        with nc.allow_low_precision(reason="qkv bf16 store"):
                            nc.vector.tensor_scalar_add(
                                dst[b][:, bc_i * TB : (bc_i + 1) * TB], acc[:], bias[:]
                            )

            def emit_vtrans(rep, b, kts=None):
                vsb = v_sb[b]
                vts = []
                for kt in kts if kts is not None else range(NKT):
                    vp = pmm.tile([128, 128], DT.bfloat16, tag="pmm", name=f"{rep}_vp{b}_{kt}")
                    nc.tensor.transpose(
                        vp[:], vsb[:, kt * 128 : (kt + 1) * 128], ident[:]
                    )
                    vt = vtp.tile([128, 130], DT.bfloat16, tag="vt", name=f"{rep}_vt{b}_{kt}")
                    nc.vector.tensor_copy(vt[:, 0:64], vp[:, 0:64])
                    nc.vector.tensor_copy(vt[:, 65:129], vp[:, 64:128])
                    with nc.allow_low_precision(reason="ones bf16"):
                        nc.vector.tensor_copy(vt[:, 64:130:65], ones2[:])
                    vts.append(vt)
                return vts

            def emit_attn_qb(rep, b, qb, vts):
                base = 0
                qsb, ksb = q_sb[b], k_sb[b]
                if True:
                    qoff = qb * QB
                    out_row = b * S + qb * QB
                    # both heads' ctx accumulators side by side in one
                    # 2-bank PSUM tile so the later copy/recip are 1 op each
                    cps_t = cps.tile([65, 2 * QB], DT.float32, tag="ctx", name=f"{rep}_c{b}_{qb}")
                    for kt in range(NKT):
                        sp = sps.tile([128, 2 * QB], DT.float32, tag="sps", name=f"{rep}_s{b}_{qb}_{kt}")
                        for h in range(2):
                            hp = slice(h * 64, (h + 1) * 64)
                            nc.tensor.matmul(
                                sp[:, h * QB : (h + 1) * QB],
                                ksb[hp, base + kt * 128 : base + (kt + 1) * 128],
                                qsb[hp, qoff : qoff + QB],
                                start=True,
                                stop=True,
                            )
                        es = esp.tile([128, 2 * QB], DT.bfloat16, tag="es", name=f"{rep}_e{b}_{qb}_{kt}")
                        ebias = mb[:, b, kt : kt + 1] if with_mask else 0.0
                        # one ScalarE op covering both heads (ACT is the
                        # bottleneck engine: amortize the ~352-cycle startup)
                        with nc.allow_low_precision(reason="exp bf16 out"):
                            nc.scalar.activation(
                                es[:], sp[:], AF.Exp, bias=ebias, scale=0.125
                            )
                        for h in range(2):
                            hs = slice(h * QB, (h + 1) * QB)
                            nc.tensor.matmul(
                                cps_t[:, hs],
                                vts[kt][:, h * 65 : (h + 1) * 65],
                                es[:, hs],
                                start=(kt == 0),
                                stop=(kt == NKT - 1),
                            )
                    # normalize -> ctxn [128 dv, QB] fp32r. Copy ctx PSUM
                    # out via one DVE op first, so the banks free for the
                    # next q-block without waiting on the whole
                    # recip -> broadcast -> mul chain.
                    ctxn = cnp.tile([128, QB], DT.bfloat16, tag="cn", name=f"{rep}_n{b}_{qb}")
                    cs = smp.tile([65, 2 * QB], DT.float32, tag="cs", name=f"{rep}_cs{b}_{qb}")
                    nc.vector.tensor_copy(cs[:], cps_t[:])
                    rr = smp.tile([1, 2 * QB], DT.float32r, tag="rr", name=f"{rep}_r{b}_{qb}")
                    with nc.allow_low_precision(reason="softmax reciprocal fp32r"):
                        nc.vector.reciprocal(rr[:], cs[64:65, :])
                    # both heads' reciprocal broadcasts accumulated into ONE
                    # PSUM bank via complementary zero-masked ones vectors:
                    # h0 writes rows 0-63 (+0 elsewhere), h1 adds rows 64-127
                    bc2 = pmm.tile([128, QB], DT.float32, tag="pmm", name=f"{rep}_bc{b}_{qb}")
                    for h in range(2):
                        hs = slice(h * QB, (h + 1) * QB)
                        nc.tensor.matmul(
                            bc2[:], onesh[:, h, :], rr[:, hs],
                            start=(h == 0), stop=(h == 1),
                        )
                    for h in range(2):
                        hs = slice(h * QB, (h + 1) * QB)
                        with nc.allow_low_precision(reason="ctx normalize to fp32r"):
                            nc.vector.tensor_mul(
                                ctxn[h * 64 : (h + 1) * 64, :], cs[0:64, hs],
                                bc2[h * 64 : (h + 1) * 64, :],
                            )
                    # output projection for this q block
                    for tt in range(QB // 128):
                        ost = osp.tile([128, 1024], DT.bfloat16, tag="os", name=f"{rep}_q{b}_{qb}_{tt}")
                        for ob in range(2):
                            op = pmm.tile(
                                [128, 512], DT.float32, tag="pmm", name=f"{rep}_o{b}_{qb}_{tt}_{ob}"
                            )
                            nc.tensor.matmul(
                                op[:],
                                ctxn[:, tt * 128 : (tt + 1) * 128],
                                wo[:, ob * 512 : (ob + 1) * 512],
                                start=True,
                                stop=True,
                            )
                            # DVE not ScalarE: ACT is the bottleneck engine;
                            # bf16 out halves DVE copy, DMA, and host readback
                            with nc.allow_low_precision(reason="bf16 partial out"):
                                nc.vector.tensor_copy(
                                    ost[:, ob * 512 : (ob + 1) * 512], op[:]
                                )
                        nc.sync.dma_start(
                            out_d[out_row + tt * 128 : out_row + (tt + 1) * 128, :],
                            ost[:],
                        )

            if phase == "full":
                # Interleave: QKV token-blocks of the NEXT batch are emitted
                # 1:1 between attention q-blocks of the current batch, so the
                # PE has independent work whenever attention stalls on
                # ScalarE exp or DVE normalize.
                seq = [(rep, b) for rep in range(reps) for b in range(B)]
                emit_qkv(*seq[0])
                vts = emit_vtrans(*seq[0])
                for i, (rep, b) in enumerate(seq):
                    nxt = seq[i + 1] if i + 1 < len(seq) else None
                    for qb in range(S // QB):
                        emit_attn_qb(rep, b, qb, vts)
                        # spread next batch's QKV one token-block per
                        # q-block, pipelining its V-transposes right after
                        # each source block, so attn(b+1) starts with no
                        # inter-batch gap and the PE filler is even
                        if nxt is not None:
                            emit_qkv(*nxt, blocks=[qb])
                            vts_next = (vts_next if qb else []) + emit_vtrans(
                                *nxt, kts=range(4 * qb, 4 * qb + 4)
                            )
                    if nxt is not None:
                        vts = vts_next
            else:
                for rep in range(reps):
                    for b in range(B):
                        emit_qkv(rep, b)
                    if phase == "qkv":
                        for i, src in enumerate((q_sb, k_sb, v_sb)):
                            for bb in range(B):
                                nc.sync.dma_start(
                                    dbg_d[i, :, bb * S : (bb + 1) * S],
                                    src[bb][:])
                    for b in range(nb if phase == "attn" else 0):
                        vts = emit_vtrans(rep, b)
                        for qb in range(S // QB):
                            emit_attn_qb(rep, b, qb, vts)
    nc.compile()
    return nc


def _make_runner(nc):
    """jit-compiled shard-mapped executor over the 8 cores, no donation so
    device-resident inputs can be reused across timed calls."""
    import jax
    from jax.experimental.shard_map import shard_map
    from jax.sharding import Mesh, NamedSharding, PartitionSpec

    from concourse import bass2jax as b2j

    b2j.install_neuronx_cc_hook()
    partition_name = nc.partition_id_tensor.name if nc.partition_id_tensor else None
    in_names, out_names, out_avals = [], [], []
    for alloc in nc.m.functions[0].allocations:
        if not isinstance(alloc, mybir.MemoryLocationSet):
            continue
        name = alloc.memorylocations[0].name
        if alloc.kind == "ExternalInput":
            if name != partition_name:
                in_names.append(name)
        elif alloc.kind == "ExternalOutput":
            out_names.append(name)
            out_avals.append(
                jax.core.ShapedArray(tuple(alloc.tensor_shape), DT.np(alloc.dtype))
            )
    n_params = len(in_names)
    all_in_names = list(in_names + out_names)
    if partition_name is not None:
        all_in_names.append(partition_name)

    def _body(*args):
        operands = list(args)
        if partition_name is not None:
            operands.append(b2j.partition_id_tensor())
        outs = b2j._bass_exec_p.bind(
            *operands,
            out_avals=tuple(out_avals),
            in_names=tuple(all_in_names),
            out_names=tuple(out_names),
            lowering_input_output_aliases=(),
            sim_require_finite=True,
            sim_require_nnan=True,
            nc=nc,
        )
        return tuple(outs)

    devices = jax.devices()[:NCORES]
    mesh = Mesh(np.asarray(devices), ("core",))
    spec = PartitionSpec("core")
    n_outs = len(out_names)
    fn = jax.jit(
        shard_map(
            _body,
            mesh=mesh,
            in_specs=(spec,) * (n_params + n_outs),
            out_specs=(spec,) * n_outs,
            check_rep=False,
        ),
        keep_unused=True,
    )

    def _body_chain(n):
        def run(*args):
            ins = args[:n_params]
            outs = tuple(args[n_params:])
            for _ in range(n):
                outs = _body(*ins, *outs)
            return outs

        return run

    def chain_fn(n):
        return jax.jit(
            shard_map(
                _body_chain(n),
                mesh=mesh,
                in_specs=(spec,) * (n_params + n_outs),
                out_specs=(spec,) * n_outs,
                check_rep=False,
            ),
            keep_unused=True,
        )

    sharding = NamedSharding(mesh, spec)

    def put(in_maps):
        concat = [
            np.concatenate([np.asarray(m[name]) for m in in_maps], axis=0)
            for name in in_names
        ]
        zeros = [
            np.zeros((NCORES * a.shape[0], *a.shape[1:]), a.dtype) for a in out_avals
        ]
        return [jax.device_put(a, sharding) for a in (*concat, *zeros)]

    fn.chain_fn = chain_fn
    return fn, put, out_names, out_avals


def _in_maps(x, attention_mask, Wq, bq, Wk, bk, Wv, bv, Wo, with_mask):
    bf16 = DT.np(DT.bfloat16)
    x = np.ascontiguousarray(np.asarray(x, dtype=np.float32))
    xT = np.ascontiguousarray(x.reshape(T, D).T).astype(bf16)  # (D, T)
    ident = np.eye(128, dtype=np.float32).astype(bf16)
    in_maps = []
    for c in range(NCORES):
        r = slice(c * DPC, (c + 1) * DPC)
        m = {
            "xT": xT,
            "wqT": np.ascontiguousarray(np.asarray(Wq, np.float32)[r, :].T).astype(bf16),
            "wkT": np.ascontiguousarray(np.asarray(Wk, np.float32)[r, :].T).astype(bf16),
            "wvT": np.ascontiguousarray(np.asarray(Wv, np.float32)[r, :].T).astype(bf16),
            "woT": np.ascontiguousarray(np.asarray(Wo, np.float32)[:, r].T).astype(bf16),
            "bq": np.asarray(bq, np.float32)[r].reshape(DPC, 1),
            "bk": np.asarray(bk, np.float32)[r].reshape(DPC, 1),
            "bv": np.asarray(bv, np.float32)[r].reshape(DPC, 1),
            "ident": ident,
        }
        if with_mask:
            mask = np.asarray(attention_mask)
            mbias = np.where(mask == 0, np.float32(-1e30), np.float32(0.0)).astype(
                np.float32
            )
            m["mbias"] = np.ascontiguousarray(mbias.reshape(B, NKT, 128))
        in_maps.append(m)
    return in_maps


def _prepare(x, attention_mask, Wq, bq, Wk, bk, Wv, bv, Wo, bo):
    """Build (cached), upload inputs, return (fn, dev_args, out_names)."""
    mask = np.asarray(attention_mask)
    with_mask = not bool((mask != 0).all())
    key = ("runner", with_mask)
    if key not in _cache:
        nc = _build(with_mask)
        _cache[key] = _make_runner(nc)
    fn, put, out_names, out_avals = _cache[key]
    dev_args = put(
        _in_maps(x, attention_mask, Wq, bq, Wk, bk, Wv, bv, Wo, with_mask)
    )
    return fn, dev_args, out_names


def kernel(x, attention_mask, Wq, bq, Wk, bk, Wv, bv, Wo, bo):
    fn, dev_args, out_names = _prepare(
        x, attention_mask, Wq, bq, Wk, bk, Wv, bv, Wo, bo
    )
    outs = fn(*dev_args)
    out_global = np.asarray(outs[out_names.index("out")])  # (8*T, D)
    acc = out_global.reshape(NCORES, T, D).astype(np.float32).sum(axis=0, dtype=np.float32)
    acc += np.asarray(bo, np.float32)[None, :]
    return acc.reshape(B, S, D)



# revision 7
# speedup vs baseline: 1.2134x; 1.2134x over previous
"""BERT self-attention (B=4, S=2048, D=1024, H=16) on 8 Trainium2 NeuronCores.

Tensor-parallel (Megatron) over heads: core c owns heads 2c, 2c+1.
  - Wq/Wk/Wv column-sharded (128 output dims per core), Wo row-sharded.
  - Each core consumes the full x, produces a partial (8192, 1024) output;
    partials are summed on the host (the Wo contraction over d_model is
    split across cores), plus bo.

The kernel is PE-stream-bound; HW microbenchmarks showed:
  - matmul cost ~ N (moving free size) cycles at up to 2.4 GHz, with K<=64
    matmuls at base partitions 0/64 co-running in separate PE row tiles
    (a K=64 N=512 pair measured 91ns vs 242ns for one K=128 N=512 matmul)
  - exp on ACT is ~0.32ns/elem (not a bottleneck)
So every matmul is shaped to minimize streamed columns:
  - QKV projections: 16 K=64 chunks as 8 co-running pairs into two PSUM
    accumulators, merged + biased by one DVE scalar_tensor_tensor.
  - scores.T [ktok 128, q 512] per head: K=64 pair (heads at partitions
    0/64) -> two 1-bank PSUM tiles; exp per head on ACT -> es bf16.
  - ctx accumulated Q-MAJOR: out [q 128, dv 64+1] per (qtile, head):
    lhsT = es slice [128 ktok, 128 q] (full M=128), rhs = vt[:, h*65:+65]
    = [V_h | ones] so N=65 and the softmax denominator accumulates in
    col 64 for free. Half the PE cost of the dv-major form (M=65).
  - normalize = DVE reciprocal of the two denom cols + 2 tensor_scalar
    muls (per-partition scalars; no PE broadcast matmul needed).
  - ctxn [q, dv] is PE-transposed to dv-major for the output projection;
    PSUM->SBUF moves ride on ACT (cheap) to keep DVE light.
Schedule: per q-block of batch b, the QKV projections + V-transposes of
batch b+1 are spread across the 16 kt iterations at fixed slots so the
PE never idles (keeps the PE p-state ramped at full clock).
"""
import sys

if "/opt/trn_rl_repo" not in sys.path:
    sys.path.insert(0, "/opt/trn_rl_repo")

import numpy as np

import concourse.bacc as bacc
import concourse.mybir as mybir
import concourse.tile as tile
from concourse.bass_utils import run_bass_kernel_spmd

DT = mybir.dt
AF = mybir.ActivationFunctionType
ALU = mybir.AluOpType

B, S, D, H = 4, 2048, 1024, 16
DK = D // H  # 64
NCORES = 8
HPC = H // NCORES  # heads per core = 2
DPC = HPC * DK  # output dims per core = 128
T = B * S  # 8192 tokens
TB = 512  # token block for projections
QB = 512  # query block for attention
NKT = S // 128  # 16 key tiles per sequence
NDT = D // 128  # 8 contraction tiles for projections

_cache = {}


def _build(with_mask, phase="full", nb=B, reps=1):
    nc = bacc.Bacc("TRN2", target_bir_lowering=False, debug=False)
    xT_d = nc.declare_dram_parameter("xT", [D, T], DT.bfloat16, isOutput=False)
    wq_d = nc.declare_dram_parameter("wqT", [D, DPC], DT.bfloat16, isOutput=False)
    wk_d = nc.declare_dram_parameter("wkT", [D, DPC], DT.bfloat16, isOutput=False)
    wv_d = nc.declare_dram_parameter("wvT", [D, DPC], DT.bfloat16, isOutput=False)
    wo_d = nc.declare_dram_parameter("woT", [DPC, D], DT.bfloat16, isOutput=False)
    bq_d = nc.declare_dram_parameter("bq", [DPC, 1], DT.float32, isOutput=False)
    bk_d = nc.declare_dram_parameter("bk", [DPC, 1], DT.float32, isOutput=False)
    bv_d = nc.declare_dram_parameter("bv", [DPC, 1], DT.float32, isOutput=False)
    id_d = nc.declare_dram_parameter("ident", [128, 128], DT.bfloat16, isOutput=False)
    if with_mask:
        mb_d = nc.declare_dram_parameter("mbias", [B, NKT, 128], DT.float32, isOutput=False)
    out_d = nc.declare_dram_parameter("out", [T, D], DT.bfloat16, isOutput=True)

    with tile.TileContext(nc) as tc:
        with (
            tc.tile_pool(name="cst", bufs=1) as cst,
            tc.tile_pool(name="qkv", bufs=1) as qkv,
            tc.tile_pool(name="xt", bufs=32) as xtp,
            tc.tile_pool(name="vt", bufs=32) as vtp,
            tc.tile_pool(name="es", bufs=4) as esp,
            tc.tile_pool(name="cn", bufs=6) as cnp,
            tc.tile_pool(name="cd", bufs=6) as cdp,
            tc.tile_pool(name="os", bufs=4) as osp,
            tc.tile_pool(name="sm", bufs=8) as smp,
            tc.tile_pool(name="sps", bufs=2, space="PSUM") as sps,
            tc.tile_pool(name="cps", bufs=1, space="PSUM") as cps,
            tc.tile_pool(name="pmm", bufs=2, space="PSUM") as pmm,
        ):
            # ---- constants / weights ----
            wq = cst.tile([128, NDT, DPC], DT.bfloat16, tag="wq")
            wk = cst.tile([128, NDT, DPC], DT.bfloat16, tag="wk")
            wv = cst.tile([128, NDT, DPC], DT.bfloat16, tag="wv")
            nc.sync.dma_start(wq[:], wq_d.rearrange("(a p) m -> p a m", p=128))
            nc.sync.dma_start(wk[:], wk_d.rearrange("(a p) m -> p a m", p=128))
            nc.sync.dma_start(wv[:], wv_d.rearrange("(a p) m -> p a m", p=128))
            wo = cst.tile([DPC, D], DT.bfloat16, tag="wo")
            nc.sync.dma_start(wo[:], wo_d[:])
            bq = cst.tile([DPC, 1], DT.float32, tag="bq")
            bk = cst.tile([DPC, 1], DT.float32, tag="bk")
            bv = cst.tile([DPC, 1], DT.float32, tag="bv")
            nc.sync.dma_start(bq[:], bq_d[:])
            nc.sync.dma_start(bk[:], bk_d[:])
            nc.sync.dma_start(bv[:], bv_d[:])
            ident = cst.tile([128, 128], DT.bfloat16, tag="ident")
            nc.sync.dma_start(ident[:], id_d[:])
            ones2 = cst.tile([128, 2], DT.float32, tag="ones2")
            nc.vector.memset(ones2[:], 1.0)
            if with_mask:
                mb = cst.tile([128, B, NKT], DT.float32, tag="mb")
                nc.sync.dma_start(mb[:], mb_d.rearrange("b a p -> p b a"))

            # persistent activations (dq/dk-major for Q,K; dv-major V before
            # transpose), one tile per batch
            q_sb = [qkv.tile([128, S], DT.bfloat16, tag=f"q{b}", name=f"q{b}") for b in range(B)]
            k_sb = [qkv.tile([128, S], DT.bfloat16, tag=f"k{b}", name=f"k{b}") for b in range(B)]
            v_sb = [qkv.tile([128, S], DT.bfloat16, tag=f"v{b}", name=f"v{b}") for b in range(B)]

            xts_memo = {}

            def ensure_xts(rep, b, bc_i):
                """Issue the 8 xT DMAs for token block bc_i of batch b."""
                key = (rep, b, bc_i)
                if key in xts_memo:
                    return xts_memo[key]
                tb = b * (S // TB) + bc_i
                xts = []
                for dt_i in range(NDT):
                    xt = xtp.tile([128, TB], DT.bfloat16, tag="xt", name=f"{rep}_xt{tb}_{dt_i}")
                    nc.sync.dma_start(
                        xt[:],
                        xT_d[dt_i * 128 : (dt_i + 1) * 128, tb * TB : (tb + 1) * TB],
                    )
                    xts.append(xt)
                xts_memo[key] = xts
                return xts

            def emit_proj(rep, b, bc_i, pname):
                """One projection (q/k/v) of token block bc_i: 16 K=64 chunks
                as 8 co-running pairs into two PSUM accumulators + merge."""
                w, bias, dst = {
                    "q": (wq, bq, q_sb), "k": (wk, bk, k_sb), "v": (wv, bv, v_sb)
                }[pname]
                xts = ensure_xts(rep, b, bc_i)
                tb = b * (S // TB) + bc_i
                acc0 = pmm.tile([128, TB], DT.float32, tag="pmm", name=f"{rep}_pa{pname}{tb}")
                acc1 = pmm.tile([128, TB], DT.float32, tag="pmm", name=f"{rep}_pb{pname}{tb}")
                for dt_i in range(NDT):
                    nc.tensor.matmul(
                        acc0[:], w[0:64, dt_i, :], xts[dt_i][0:64, :],
                        start=(dt_i == 0), stop=(dt_i == NDT - 1),
                        skip_group_check=True,
                    )
                    nc.tensor.matmul(
                        acc1[:], w[64:128, dt_i, :], xts[dt_i][64:128, :],
                        start=(dt_i == 0), stop=(dt_i == NDT - 1),
                        skip_group_check=True,
                    )
                # stt cannot read two PSUM operands (NCC_IBVF027): stage acc1
                # through SBUF on ACT (idle capacity), then one DVE stt.
                tmp = smp.tile([128, TB], DT.float32, tag="ptmp", name=f"{rep}_pt{pname}{tb}", bufs=2)
                nc.scalar.copy(tmp[:], acc1[:])
                with nc.allow_low_precision(reason="qkv bf16 store"):
                    nc.vector.scalar_tensor_tensor(
                        dst[b][:, bc_i * TB : (bc_i + 1) * TB],
                        acc0[:], bias[:], tmp[:], ALU.add, ALU.add,
                    )

            def emit_vtrans(rep, b, kts):
                """PE-transpose 128-token V tiles into [tok 128, 130] =
                [V_h0 64 | ones | V_h1 64 | ones] (matmul rhs + denom)."""
                vsb = v_sb[b]
                vts = []
                for kt in kts:
                    vp = pmm.tile([128, 128], DT.bfloat16, tag="pmm", name=f"{rep}_vp{b}_{kt}")
                    nc.tensor.transpose(
                        vp[:], vsb[:, kt * 128 : (kt + 1) * 128], ident[:]
                    )
                    vt = vtp.tile([128, 130], DT.bfloat16, tag="vt", name=f"{rep}_vt{b}_{kt}")
                    nc.vector.tensor_copy(vt[:, 0:64], vp[:, 0:64])
                    nc.vector.tensor_copy(vt[:, 65:129], vp[:, 64:128])
                    with nc.allow_low_precision(reason="ones bf16"):
                        nc.vector.tensor_copy(vt[:, 64:130:65], ones2[:])
                    vts.append(vt)
                return vts

            def emit_scores_exp(rep, b, qb, kt):
                """co-running K=64 scores pair -> one joint exp on ACT."""
                qsb, ksb = q_sb[b], k_sb[b]
                qoff = qb * QB
                sp = sps.tile([128, 2 * QB], DT.float32, tag="sp", name=f"{rep}_s{b}_{qb}_{kt}")
                for h in range(2):
                    hp = slice(h * 64, (h + 1) * 64)
                    nc.tensor.matmul(
                        sp[:, h * QB : (h + 1) * QB],
                        ksb[hp, kt * 128 : (kt + 1) * 128],
                        qsb[hp, qoff : qoff + QB],
                        start=True, stop=True,
                    )
                ebias = mb[:, b, kt : kt + 1] if with_mask else 0.0
                es = esp.tile([128, 2 * QB], DT.bfloat16, tag="es", name=f"{rep}_e{b}_{qb}_{kt}")
                with nc.allow_low_precision(reason="exp bf16 out"):
                    nc.scalar.activation(es[:], sp[:], AF.Exp, bias=ebias, scale=0.125)
                return es

            def emit_ctx(rep, b, qb, kt, es, vts, cps_t3, cps_t1):
                """8 q-major ctx matmuls: out [q 128, V_h 64 | denom 1].

                PSUM bank rule (measured on HW): start=True zeroes the WHOLE
                bank, so each bank is reset exactly once per q-block (first
                matmul into it) and every other matmul accumulates."""
                for qt in range(4):
                    ct = cps_t3[:, qt * 130 : (qt + 1) * 130] if qt < 3 else cps_t1[:]
                    first_of_bank = (qt == 0) or (qt == 3)
                    last_of_bank = (qt == 2) or (qt == 3)
                    for h in range(2):
                        nc.tensor.matmul(
                            ct[:, h * 65 : (h + 1) * 65],
                            es[:, h * QB + qt * 128 : h * QB + (qt + 1) * 128],
                            vts[kt][:, h * 65 : (h + 1) * 65],
                            start=(kt == 0 and first_of_bank and h == 0),
                            stop=(kt == NKT - 1 and last_of_bank and h == 1),
                            skip_group_check=True,
                        )

            def emit_boundary(rep, b, qb, cps_t3, cps_t1):
                """normalize -> transpose to dv-major -> output projection."""
                out_row = b * S + qb * QB
                ctxds = []
                for qt in range(4):
                    ct = cps_t3[:, qt * 130 : (qt + 1) * 130] if qt < 3 else cps_t1[:]
                    rr = smp.tile([128, 2], DT.float32, tag="rr", name=f"{rep}_r{b}_{qb}_{qt}")
                    nc.vector.reciprocal(rr[:], ct[:, 64:130:65])
                    ctxn = cnp.tile([128, 128], DT.bfloat16, tag="cn", name=f"{rep}_n{b}_{qb}_{qt}")
                    with nc.allow_low_precision(reason="ctx normalize bf16"):
                        nc.vector.tensor_scalar_mul(ctxn[:, 0:64], ct[:, 0:64], rr[:, 0:1])
                        nc.vector.tensor_scalar_mul(ctxn[:, 64:128], ct[:, 65:129], rr[:, 1:2])
                    tp = pmm.tile([128, 128], DT.bfloat16, tag="pmm", name=f"{rep}_t{b}_{qb}_{qt}")
                    nc.tensor.transpose(tp[:], ctxn[:], ident[:])
                    ctxd = cdp.tile([128, 128], DT.bfloat16, tag="cd", name=f"{rep}_d{b}_{qb}_{qt}")
                    with nc.allow_low_precision(reason="bf16 copy"):
                        nc.scalar.copy(ctxd[:], tp[:])
                    ctxds.append(ctxd)
                for qt in range(4):
                    ost = osp.tile([128, D], DT.bfloat16, tag="os", name=f"{rep}_o{b}_{qb}_{qt}")
                    for ob in range(2):
                        op = pmm.tile([128, 512], DT.float32, tag="pmm", name=f"{rep}_op{b}_{qb}_{qt}_{ob}")
                        nc.tensor.matmul(
                            op[:], ctxds[qt][:], wo[:, ob * 512 : (ob + 1) * 512],
                            start=True, stop=True,
                        )
                        with nc.allow_low_precision(reason="bf16 partial out"):
                            nc.scalar.copy(ost[:, ob * 512 : (ob + 1) * 512], op[:])
                    nc.sync.dma_start(
                        out_d[out_row + qt * 128 : out_row + (qt + 1) * 128, :],
                        ost[:],
                    )

            assert phase == "full"
            seq = [(rep, b) for rep in range(reps) for b in range(B)]
            # prologue: QKV + vtrans of the first batch
            rep0, b0 = seq[0]
            ensure_xts(rep0, b0, 0)
            ensure_xts(rep0, b0, 1)
            for bc_i in range(S // TB):
                if bc_i + 2 < S // TB:
                    ensure_xts(rep0, b0, bc_i + 2)
                for pname in ("q", "k", "v"):
                    emit_proj(rep0, b0, bc_i, pname)
            vts = emit_vtrans(rep0, b0, range(NKT))

            for i, (rep, b) in enumerate(seq):
                nxt = seq[i + 1] if i + 1 < len(seq) else None
                if nxt is not None:
                    ensure_xts(*nxt, 0)
                    ensure_xts(*nxt, 1)
                vts_next = []
                # flat (qb, kt) pipeline: ctx lags scores/exp by one step so
                # the PE never waits on ACT; the previous q-block's boundary
                # (normalize/transpose/outproj) is emitted right after the
                # next q-block's first scores so PE filler covers the DVE
                # normalize latency.
                cts = {}

                def cps_of(qb):
                    if qb not in cts:
                        cts[qb] = (
                            cps.tile([128, 390], DT.float32, tag="c3", name=f"{rep}_c3_{b}_{qb}"),
                            cps.tile([128, 130], DT.float32, tag="c1", name=f"{rep}_c1_{b}_{qb}"),
                        )
                    return cts[qb]

                items = [(qb, kt) for qb in range(S // QB) for kt in range(NKT)]
                prev = None
                for qb, kt in items:
                    es = emit_scores_exp(rep, b, qb, kt)
                    if prev is not None:
                        pqb, pkt, pes = prev
                        emit_ctx(rep, b, pqb, pkt, pes, vts, *cps_of(pqb))
                    if kt == 0 and qb > 0:
                        emit_boundary(rep, b, qb - 1, *cps_of(qb - 1))
                    prev = (qb, kt, es)
                    if nxt is not None:
                        if kt == 2:
                            emit_proj(*nxt, qb, "q")
                        elif kt == 5:
                            emit_proj(*nxt, qb, "k")
                        elif kt == 8:
                            emit_proj(*nxt, qb, "v")
                        elif kt == 10:
                            vts_next += emit_vtrans(*nxt, [4 * qb, 4 * qb + 1])
                        elif kt == 12:
                            vts_next += emit_vtrans(*nxt, [4 * qb + 2, 4 * qb + 3])
                        elif kt == 14 and qb + 2 < S // TB:
                            ensure_xts(*nxt, qb + 2)
                pqb, pkt, pes = prev
                emit_ctx(rep, b, pqb, pkt, pes, vts, *cps_of(pqb))
                emit_boundary(rep, b, pqb, *cps_of(pqb))
                if nxt is not None:
                    vts = vts_next
    nc.compile()
    return nc


def _make_runner(nc):
    """jit-compiled shard-mapped executor over the 8 cores, no donation so
    device-resident inputs can be reused across timed calls."""
    import jax
    from jax.experimental.shard_map import shard_map
    from jax.sharding import Mesh, NamedSharding, PartitionSpec

    from concourse import bass2jax as b2j

    b2j.install_neuronx_cc_hook()
    partition_name = nc.partition_id_tensor.name if nc.partition_id_tensor else None
    in_names, out_names, out_avals = [], [], []
    for alloc in nc.m.functions[0].allocations:
        if not isinstance(alloc, mybir.MemoryLocationSet):
            continue
        name = alloc.memorylocations[0].name
        if alloc.kind == "ExternalInput":
            if name != partition_name:
                in_names.append(name)
        elif alloc.kind == "ExternalOutput":
            out_names.append(name)
            out_avals.append(
                jax.core.ShapedArray(tuple(alloc.tensor_shape), DT.np(alloc.dtype))
            )
    n_params = len(in_names)
    all_in_names = list(in_names + out_names)
    if partition_name is not None:
        all_in_names.append(partition_name)

    def _body(*args):
        operands = list(args)
        if partition_name is not None:
            operands.append(b2j.partition_id_tensor())
        outs = b2j._bass_exec_p.bind(
            *operands,
            out_avals=tuple(out_avals),
            in_names=tuple(all_in_names),
            out_names=tuple(out_names),
            lowering_input_output_aliases=(),
            sim_require_finite=True,
            sim_require_nnan=True,
            nc=nc,
        )
        return tuple(outs)

    devices = jax.devices()[:NCORES]
    mesh = Mesh(np.asarray(devices), ("core",))
    spec = PartitionSpec("core")
    n_outs = len(out_names)
    fn = jax.jit(
        shard_map(
            _body,
            mesh=mesh,
            in_specs=(spec,) * (n_params + n_outs),
            out_specs=(spec,) * n_outs,
            check_rep=False,
        ),
        keep_unused=True,
    )

    sharding = NamedSharding(mesh, spec)

    def put(in_maps):
        concat = [
            np.concatenate([np.asarray(m[name]) for m in in_maps], axis=0)
            for name in in_names
        ]
        zeros = [
            np.zeros((NCORES * a.shape[0], *a.shape[1:]), a.dtype) for a in out_avals
        ]
        return [jax.device_put(a, sharding) for a in (*concat, *zeros)]

    return fn, put, out_names, out_avals


def _in_maps(x, attention_mask, Wq, bq, Wk, bk, Wv, bv, Wo, with_mask):
    bf16 = DT.np(DT.bfloat16)
    x = np.ascontiguousarray(np.asarray(x, dtype=np.float32))
    xT = np.ascontiguousarray(x.reshape(T, D).T).astype(bf16)  # (D, T)
    ident = np.eye(128, dtype=np.float32).astype(bf16)
    in_maps = []
    for c in range(NCORES):
        r = slice(c * DPC, (c + 1) * DPC)
        m = {
            "xT": xT,
            "wqT": np.ascontiguousarray(np.asarray(Wq, np.float32)[r, :].T).astype(bf16),
            "wkT": np.ascontiguousarray(np.asarray(Wk, np.float32)[r, :].T).astype(bf16),
            "wvT": np.ascontiguousarray(np.asarray(Wv, np.float32)[r, :].T).astype(bf16),
            "woT": np.ascontiguousarray(np.asarray(Wo, np.float32)[:, r].T).astype(bf16),
            "bq": np.asarray(bq, np.float32)[r].reshape(DPC, 1),
            "bk": np.asarray(bk, np.float32)[r].reshape(DPC, 1),
            "bv": np.asarray(bv, np.float32)[r].reshape(DPC, 1),
            "ident": ident,
        }
        if with_mask:
            mask = np.asarray(attention_mask)
            mbias = np.where(mask == 0, np.float32(-1e30), np.float32(0.0)).astype(
                np.float32
            )
            m["mbias"] = np.ascontiguousarray(mbias.reshape(B, NKT, 128))
        in_maps.append(m)
    return in_maps


def _prepare(x, attention_mask, Wq, bq, Wk, bk, Wv, bv, Wo, bo):
    """Build (cached), upload inputs, return (fn, dev_args, out_names)."""
    mask = np.asarray(attention_mask)
    with_mask = not bool((mask != 0).all())
    key = ("runner", with_mask)
    if key not in _cache:
        nc = _build(with_mask)
        _cache[key] = _make_runner(nc)
    fn, put, out_names, out_avals = _cache[key]
    dev_args = put(
        _in_maps(x, attention_mask, Wq, bq, Wk, bk, Wv, bv, Wo, with_mask)
    )
    return fn, dev_args, out_names


def kernel(x, attention_mask, Wq, bq, Wk, bk, Wv, bv, Wo, bo):
    fn, dev_args, out_names = _prepare(
        x, attention_mask, Wq, bq, Wk, bk, Wv, bv, Wo, bo
    )
    outs = fn(*dev_args)
    out_global = np.asarray(outs[out_names.index("out")])  # (8*T, D)
    acc = out_global.reshape(NCORES, T, D).astype(np.float32).sum(axis=0, dtype=np.float32)
    acc += np.asarray(bo, np.float32)[None, :]
    return acc.reshape(B, S, D)
